# revision 1
# baseline (speedup 1.0000x reference)
"""ChannelBlockImportanceGate kernel for 8 Trainium2 NeuronCores.

Computes, per (b, c) slice of features [8, 256, 132, 132] f32:
  scores = block-sum of |x| over 8x8 blocks (17x17 grid, zero-padded edges)
  top-72 blocks (ties -> lowest index, matching jax.lax.top_k)
  output = per-pixel {0,1} mask upsampled 8x8 (cropped to 132x132)

The straight-through soft term of the reference cancels in the forward
pass (hard - sg(soft) + soft == hard up to ~1ulp), so the output is the
hard mask.

Sharding: purely data parallel. 2048 (b,c) slices -> 256 per core.
Per core: 2 groups of 128 slices; each slice occupies one SBUF
partition so pooling/topk/upsample are per-partition ops with no
cross-partition traffic. Top-72 uses 9 rounds of DVE max8 +
match_replace(-1e30), then mask = (score < 0).
"""

import numpy as np

B, C, H, W = 8, 256, 132, 132
HW = H * W            # 17424
NB = 17               # 8x8 blocks per side (132 padded to 136)
NBLK = NB * NB        # 289
KEEP = 72             # round(289 * 0.25)
N_CORES = 8
S = (B * C) // N_CORES  # 256 slices per core
ROW_CHUNKS = ((0, 32), (32, 64), (64, 96), (96, 132))
NEG = -1.0e30

_prog_cache = {}


def _build_program():
    import concourse.bacc as bacc
    import concourse.mybir as mybir
    import concourse.tile as tile

    f32 = mybir.dt.float32
    X = mybir.AxisListType.X
    ADD = mybir.AluOpType.add

    nc = bacc.Bacc("TRN2", debug=False, num_devices=N_CORES)
    x = nc.dram_tensor("x", (S, HW), f32, kind="ExternalInput")
    y = nc.dram_tensor("y", (S, HW), f32, kind="ExternalOutput")

    with tile.TileContext(nc) as tc:
        with (
            tc.tile_pool(name="big", bufs=2) as bigp,
            tc.tile_pool(name="med", bufs=2) as medp,
            tc.tile_pool(name="small", bufs=2) as smallp,
        ):
            for g in range(S // 128):
                p0 = g * 128
                chunks = []
                for k, (r0, r1) in enumerate(ROW_CHUNKS):
                    ch = bigp.tile([128, (r1 - r0) * W], f32,
                                   name=f"ch_g{g}k{k}", tag=f"chunk{k}")
                    nc.sync.dma_start(out=ch[:, :],
                                      in_=x[p0:p0 + 128, r0 * W:r1 * W])
                    chunks.append(ch)

                # W-pool: per image row, |x| summed over 8-col groups
                # (16 full groups + one 4-col partial group).
                wsum = medp.tile([128, H * NB], f32,
                                 name=f"wsum_g{g}", tag="wsum")
                ws3 = wsum.rearrange("p (r t) -> p r t", t=NB)
                for k, (r0, r1) in enumerate(ROW_CHUNKS):
                    v = chunks[k].rearrange("p (r w) -> p r w", w=W)
                    nc.vector.tensor_reduce(
                        out=ws3[:, r0:r1, 0:16],
                        in_=v[:, :, 0:128].rearrange("p r (q c) -> p r q c", c=8),
                        axis=X, op=ADD, apply_absolute_value=True)
                    nc.vector.tensor_reduce(
                        out=ws3[:, r0:r1, 16:17],
                        in_=v[:, :, 128:132],
                        axis=X, op=ADD, apply_absolute_value=True)

                # H-pool: row sums summed over 8-row groups (16 full + 4-row
                # partial) -> scores [128, 289], layout h*17 + w.
                scores = smallp.tile([128, NBLK], f32,
                                     name=f"scores_g{g}", tag="scores")
                sc3 = scores.rearrange("p (h t) -> p h t", t=NB)
                nc.vector.tensor_reduce(
                    out=sc3[:, 0:16, :],
                    in_=ws3[:, 0:128, :].rearrange("p (h r) t -> p h t r", r=8),
                    axis=X, op=ADD)
                nc.vector.tensor_reduce(
                    out=sc3[:, 16:17, :],
                    in_=ws3[:, 128:132, :].rearrange("p r t -> p t r"),
                    axis=X, op=ADD)

                # Top-72 per partition: 9 rounds of max8 + match_replace.
                # match_replace replaces the first unmatched occurrence, so
                # ties resolve to the lowest index like jax.lax.top_k.
                for it in range(KEEP // 8):
                    m8 = smallp.tile([128, 8], f32,
                                     name=f"m8_g{g}i{it}", tag="m8")
                    nc.vector.max(out=m8[:, :], in_=scores[:, :])
                    nc.vector.match_replace(out=scores[:, :],
                                            in_to_replace=m8[:, :],
                                            in_values=scores[:, :],
                                            imm_value=NEG)

                # Block mask: replaced entries are -1e30, real scores are >= 0.
                mask = smallp.tile([128, NBLK], f32,
                                   name=f"mask_g{g}", tag="mask")
                nc.vector.tensor_scalar(out=mask[:, :], in0=scores[:, :],
                                        scalar1=0.0, scalar2=None,
                                        op0=mybir.AluOpType.is_lt)
                m3 = mask.rearrange("p (h t) -> p h t", t=NB)

                # Upsample 8x8 (broadcast copies) in place over the feature
                # chunks, then store. The bulk (region A) runs on the
                # otherwise-idle Scalar engine as 3D copies (its ISA is
                # 3D-only) so the Vector engine keeps pooling/topk of the
                # other group; only the tiny edge regions stay on Vector,
                # emitted first so the same-tile WAW ordering vs the
                # Scalar copies costs nothing.
                for k, (r0, r1) in enumerate(ROW_CHUNKS):
                    ch = chunks[k]
                    v = ch.rearrange("p (r w) -> p r w", w=W)
                    hg0 = r0 // 8
                    nfull = (min(r1, 128) - r0) // 8
                    nr = nfull * 8
                    outB = v[:, 0:nr, 128:132].rearrange(
                        "p (h r) c -> p h r c", r=8)
                    inB = (m3[:, hg0:hg0 + nfull, 16:17]
                           .unsqueeze(2).broadcast_to((128, nfull, 8, 4)))
                    nc.vector.tensor_copy(out=outB, in_=inB)
                    if r1 > 128:  # rows 128..131: the 4-row partial hgroup
                        a = 128 - r0
                        outD = v[:, a:a + 4, 128:132]
                        inD = m3[:, 16:17, 16:17].broadcast_to((128, 4, 4))
                        nc.vector.tensor_copy(out=outD, in_=inD)
                    outA = v[:, 0:nr, 0:128].rearrange(
                        "p (h r) (q c) -> p h r q c", r=8, c=8)
                    inA = (m3[:, hg0:hg0 + nfull, 0:16]
                           .unsqueeze(2).unsqueeze(4)
                           .broadcast_to((128, nfull, 8, 16, 8)))
                    for r in range(8):
                        nc.scalar.copy(out=outA[:, :, r, :, :],
                                       in_=inA[:, :, r, :, :])
                    if r1 > 128:
                        a = 128 - r0
                        outC = v[:, a:a + 4, 0:128].rearrange(
                            "p r (q c) -> p r q c", c=8)
                        inC = (m3[:, 16:17, 0:16].unsqueeze(3)
                               .broadcast_to((128, 4, 16, 8)))
                        nc.scalar.copy(out=outC, in_=inC)
                    nc.sync.dma_start(out=y[p0:p0 + 128, r0 * W:r1 * W],
                                      in_=ch[:, :])
    nc.compile()
    return nc


def _ensure_ntff_hook_module():
    """bass_utils' trace path does `from antenv.axon_hooks import
    get_axon_ntff_profile_hook` — a module this image doesn't ship.
    Register an equivalent (ctypes into libaxon_pjrt.so, mirroring
    trn_boot._ntff_profile_via_ctypes) so BASS_TRACE=1 works; degrade
    to a None hook (trace skipped) when unavailable."""
    import sys
    import types

    try:
        import antenv.axon_hooks  # noqa: F401
        return
    except Exception:
        pass

    hook = None
    try:
        import contextlib
        import ctypes

        so_path = "/opt/axon/libaxon_pjrt.so"
        lib = ctypes.CDLL(so_path)
        if hasattr(lib, "axon_start_nrt_profile"):
            lib.axon_start_nrt_profile.argtypes = [
                ctypes.POINTER(ctypes.c_int64), ctypes.c_size_t]
            lib.axon_start_nrt_profile.restype = ctypes.c_int64
            lib.axon_stop_nrt_profile.argtypes = [ctypes.c_char_p]
            lib.axon_stop_nrt_profile.restype = ctypes.c_int64

            @contextlib.contextmanager
            def _hook(output_dir, device_ids):
                import jax
                jax.devices()
                if device_ids:
                    ids = (ctypes.c_int64 * len(device_ids))(*device_ids)
                    rc = lib.axon_start_nrt_profile(ids, len(device_ids))
                else:
                    rc = lib.axon_start_nrt_profile(None, 0)
                if rc != 0:
                    raise RuntimeError(f"axon_start_nrt_profile rc={rc}")
                try:
                    yield
                finally:
                    n = lib.axon_stop_nrt_profile(str(output_dir).encode())
                    print(f"ntff profile: {n} file(s) -> {output_dir}",
                          file=sys.stderr)

            hook = _hook
    except Exception:
        hook = None

    mod = types.ModuleType("antenv.axon_hooks")
    mod.get_axon_ntff_profile_hook = lambda: hook
    mod.set_axon_ntff_profile_hook = lambda h: None
    sys.modules["antenv.axon_hooks"] = mod


def _get_program():
    if "nc" not in _prog_cache:
        _prog_cache["nc"] = _build_program()
    return _prog_cache["nc"]


def kernel(features, enabled):
    feats = np.asarray(features)
    if not bool(np.asarray(enabled)):
        return np.ones(feats.shape, dtype=np.float32)

    _ensure_ntff_hook_module()
    import concourse.bass_utils as _bu
    from concourse.bass_utils import run_bass_kernel_spmd

    # The trace path uploads artifacts to a shared bucket; tolerate
    # sandboxes where that fails.
    if not getattr(_bu, "_upload_patched", False):
        _orig_upload = _bu.upload_artifacts

        def _safe_upload(tmpdir):
            try:
                return _orig_upload(tmpdir)
            except Exception:
                return str(tmpdir)

        _bu.upload_artifacts = _safe_upload
        _bu._upload_patched = True

    nc = _get_program()
    flat = np.ascontiguousarray(feats.reshape(B * C, HW), dtype=np.float32)
    in_maps = [{"x": flat[i * S:(i + 1) * S]} for i in range(N_CORES)]
    res = run_bass_kernel_spmd(nc, in_maps, list(range(N_CORES)))
    _prog_cache["last_res"] = res
    out = np.concatenate([np.asarray(res.results[i]["y"])
                          for i in range(N_CORES)], axis=0)
    return out.reshape(B, C, H, W).astype(np.float32)

